# revision 16
# baseline (speedup 1.0000x reference)
"""ConvSTFT on Trainium2: strided conv of x[32, 480000] against a fixed
[514, 1, 400] Fourier basis, hop 100 -> out [32, 514, 4803] f32.

Sharding: pure data parallel. Batch dim (32) split 4-per-core across 8
NeuronCores; the small [514, 1, 400] Fourier weight is replicated.

Split of work: PE matmul cost is (#streams) x N cycles, with #streams =
ceil(C/128) * ceil(WIN/HOP) per frame-column. C=514 needs 5 channel
tiles, the 5th holding only 2 channels -- 25% wasted PE time. The device
computes channels 0..511 (4 full tiles = the bf16 PE floor of 16
streams/frame-column ~= 130us/core at the 2.37 GHz top pstate); the host
computes the last 2 channels with one small BLAS GEMM over the strided
frame view (<1% of the FLOPs, valid for any weight values). Device
output is stored bf16 (halves the dominant output DMA: 39.3 -> 19.7
MB/core; rel err 2.1e-3 -> 3.4e-3 vs the 2e-2 gate) and upcast on host.

Host prep: pad x by 300 on both sides, transpose per batch to
XT[b, r, f'] = x_padded[b, 100 f' + r] so device input DMAs move whole
[100, cols] panels with multi-KB contiguous lines; weights passed as
wt[j, r, c] = weight[c, 0, 100 j + r] (independent [100, 512] j-planes).

Device kernel (Bass/Tile): the t = 100j + r tap decomposition (j 0..3,
r 0..99) turns the overlapped strided conv into 4 PSUM-accumulated
matmuls per group:
    out[c, f] = sum_j sum_r wt[j, r, c] * XT[r, f + j]
lhsT = wt[j][:, c-tile] (K=100, M=128), rhs = XT[:, f-tile] (N<=512),
fp32 PSUM accumulation over j with all 8 banks in flight; PSUM is
evacuated alternately by DVE/ACT (casting to bf16) into SBUF rows
[128, 4803], stored in four column pieces (the last only ~50 KB).

Startup (all measured on this hw): engines boot staggered within a NEFF
exec (sync ~0.2us, gpsimd ~5.8, scalar/ACT ~7.1, vector/DVE ~7.4), and
first-DMA completions cannot beat ~10us (queue boot + descriptor pacing
+ completion latency). So: the first XT panel rides the sync ring, the
four weight j-planes fan out over scalar/gpsimd/gpsimd/sync, and the PE
warmup source is a DVE memset (fastest PE start, ~7.9us). The warmup
matmuls MUST be K=128 and span ~3.4us of sustained activity: K=128
activity is what trips the PE clock boost (1.2 -> 2.4 GHz); K=100 work
neither lifts nor re-lifts it, and an unramped stream runs 2.4x slower.
Once lifted, the K=100 stream holds the boost. Measured ~149.5us/core =
~11.8us startup + ~131.5us PE stream (PE >99% busy within its window) +
~3.4us store drain + ~3.2us fixed epilogue; baseline was 183.3us."""

import numpy as np
import ml_dtypes

WIN, HOP, C = 400, 100, 514
C_DEV = 512                           # channels computed on device
B, T = 32, 480000
PAD = WIN - HOP                       # 300
N_CORES = 8
B_LOC = B // N_CORES                  # 4
T_PAD = T + 2 * PAD                   # 480600
N_FRAMES = (T_PAD - WIN) // HOP + 1   # 4803
N_CHUNKS = 4864                       # padded frame columns (128-aligned)
NJ = WIN // HOP                       # 4

F_TILE = 512
C_TILE = 128
FIRST_COLS = 640                      # first XT panel (critical load)
N_WARMUP = 12
STORE_EVERY = 3                       # ftile groups per output store piece


def build_program(b_loc=B_LOC, n_chunks=N_CHUNKS, n_frames=N_FRAMES):
    import concourse.bacc as bacc
    import concourse.mybir as mybir
    import concourse.tile as tile

    dt = mybir.dt
    assert n_frames + NJ - 1 <= n_chunks

    nc = bacc.Bacc("TRN2", target_bir_lowering=False, debug=False)
    x_d = nc.dram_tensor(
        "x", [b_loc, HOP, n_chunks], dt.bfloat16, kind="ExternalInput"
    ).ap()
    w_d = nc.dram_tensor(
        "wt", [NJ, HOP, C_DEV], dt.bfloat16, kind="ExternalInput"
    ).ap()
    o_d = nc.dram_tensor(
        "out", [b_loc, C_DEV, n_frames], dt.bfloat16, kind="ExternalOutput"
    ).ap()

    ctiles = [(c0, min(C_TILE, C_DEV - c0)) for c0 in range(0, C_DEV, C_TILE)]
    ftiles = [(f0, min(F_TILE, n_frames - f0)) for f0 in range(0, n_frames, F_TILE)]
    n_ft = len(ftiles)
    store_at = {}
    lo = 0
    for fi in range(STORE_EVERY - 1, n_ft - 1, STORE_EVERY):
        hi = ftiles[fi][0] + ftiles[fi][1]
        store_at[fi] = (lo, hi)
        lo = hi
    store_at[n_ft - 1] = (lo, n_frames)

    with tile.TileContext(nc) as tc:
        with (
            tc.tile_pool(name="const", bufs=1) as constp,
            tc.tile_pool(name="xt", bufs=2) as xtp,
            tc.tile_pool(name="orow", bufs=7) as orowp,
            tc.tile_pool(name="mmps", bufs=8, space="PSUM") as mmps,
        ):
            # batch 0 rides the sync ring entirely: 5 column panels + the
            # j3 weight plane = 6 DMAs, one per sync queue, all landing by
            # ~11us. j0/j1/j2 go as row-halves so they parallelize across
            # their rings' queues.
            xt0 = xtp.tile([HOP, n_chunks], dt.bfloat16, tag="xt")
            nc.sync.dma_start(xt0[:, 0:FIRST_COLS], x_d[0, :, 0:FIRST_COLS])
            wsb = constp.tile([HOP, NJ, C_DEV], dt.bfloat16)
            nc.sync.dma_start(wsb[:, NJ - 1, :], w_d[NJ - 1])
            for g0 in range(FIRST_COLS, n_chunks, 1056):
                gs = min(1056, n_chunks - g0)
                nc.sync.dma_start(xt0[:, g0 : g0 + gs], x_d[0, :, g0 : g0 + gs])
            for j, eng in ((0, nc.scalar), (1, nc.gpsimd), (2, nc.gpsimd)):
                eng.dma_start(wsb[0:50, j, :], w_d[j, 0:50])
                eng.dma_start(wsb[50:HOP, j, :], w_d[j, 50:HOP])

            warm = constp.tile([128, 512], dt.bfloat16)
            nc.vector.memset(warm[:], 0.0)
            wps = mmps.tile([128, F_TILE], dt.float32, tag="ps")
            for _ in range(N_WARMUP):
                nc.tensor.matmul(wps[0:16, :], warm[:, 0:16], warm[:])

            ncopy = 0

            def mm_group(xt, orow, c0, cm, f0, fn):
                nonlocal ncopy
                ps = mmps.tile([128, F_TILE], dt.float32, tag="ps")
                for j in range(NJ):
                    nc.tensor.matmul(
                        ps[0:cm, 0:fn],
                        wsb[0:HOP, j, c0 : c0 + cm],
                        xt[0:HOP, f0 + j : f0 + j + fn],
                        start=(j == 0),
                        stop=(j == NJ - 1),
                    )
                if ncopy % 2 == 1:
                    nc.scalar.copy(orow[0:cm, f0 : f0 + fn], ps[0:cm, 0:fn])
                else:
                    nc.vector.tensor_copy(orow[0:cm, f0 : f0 + fn], ps[0:cm, 0:fn])
                ncopy += 1

            for b in range(b_loc):
                if b == 0:
                    xt = xt0
                else:
                    xt = xtp.tile([HOP, n_chunks], dt.bfloat16, tag="xt")
                    for g0 in range(0, n_chunks, 1216):
                        gs = min(1216, n_chunks - g0)
                        nc.scalar.dma_start(
                            xt[:, g0 : g0 + gs], x_d[b, :, g0 : g0 + gs]
                        )

                for c0, cm in ctiles:
                    orow = orowp.tile(
                        [128, n_frames], dt.bfloat16, tag="orow", name=f"or_{b}_{c0}"
                    )
                    for fi, (f0, fn) in enumerate(ftiles):
                        mm_group(xt, orow, c0, cm, f0, fn)
                        if fi in store_at:
                            slo, shi = store_at[fi]
                            nc.sync.dma_start(
                                o_d[b, c0 : c0 + cm, slo:shi], orow[0:cm, slo:shi]
                            )

    nc.compile()
    return nc


_NC = None
LAST_RESULTS = None


def _ensure_axon_hooks_stub():
    import sys

    try:
        import antenv.axon_hooks  # noqa: F401
    except ImportError:
        import types

        import antenv

        m = types.ModuleType("antenv.axon_hooks")
        m.get_axon_ntff_profile_hook = lambda: None
        m.set_axon_ntff_profile_hook = lambda h: None
        sys.modules["antenv.axon_hooks"] = m
        antenv.axon_hooks = m


def _prep_inputs(x, weight):
    x = np.asarray(x, dtype=np.float32)
    w = np.asarray(weight, dtype=np.float32)
    nb = x.shape[0]
    xp = np.zeros((nb, N_CHUNKS * HOP), dtype=np.float32)
    xp[:, PAD : PAD + x.shape[1]] = x
    xdev = np.ascontiguousarray(
        xp.reshape(nb, N_CHUNKS, HOP).transpose(0, 2, 1)
    ).astype(ml_dtypes.bfloat16)
    wt = np.ascontiguousarray(
        w.reshape(C, WIN)[:C_DEV].T.reshape(NJ, HOP, C_DEV)
    ).astype(ml_dtypes.bfloat16)
    return xp, xdev, wt


def _host_tail_channels(xp, w):
    w2 = np.ascontiguousarray(
        np.asarray(w, dtype=np.float32).reshape(C, WIN)[C_DEV:].T
    )
    v = np.lib.stride_tricks.sliding_window_view(xp, WIN, axis=1)[:, ::HOP, :]
    v = v[:, :N_FRAMES]
    out2 = np.tensordot(v, w2, axes=([2], [0]))
    return np.ascontiguousarray(out2.transpose(0, 2, 1))


def kernel(x, weight):
    global _NC, LAST_RESULTS
    from concourse.bass_utils import run_bass_kernel_spmd

    _ensure_axon_hooks_stub()
    xp, xdev, wt = _prep_inputs(x, weight)
    tail = _host_tail_channels(xp, weight)
    if _NC is None:
        _NC = build_program()
    in_maps = [
        {"x": np.ascontiguousarray(xdev[c * B_LOC : (c + 1) * B_LOC]), "wt": wt}
        for c in range(N_CORES)
    ]
    res = run_bass_kernel_spmd(_NC, in_maps, core_ids=list(range(N_CORES)))
    LAST_RESULTS = res
    out = np.empty((B, C, N_FRAMES), dtype=np.float32)
    for c in range(N_CORES):
        out[c * B_LOC : (c + 1) * B_LOC, :C_DEV] = res.results[c]["out"]
    out[:, C_DEV:] = tail
    return out


# revision 17
# speedup vs baseline: 1.0118x; 1.0118x over previous
"""ConvSTFT on Trainium2: strided conv of x[32, 480000] against a fixed
[514, 1, 400] Fourier basis, hop 100 -> out [32, 514, 4803] f32.

Sharding: pure data parallel. Batch dim (32) split 4-per-core across 8
NeuronCores; the small [514, 1, 400] Fourier weight is replicated.

Split of work: PE matmul cost is (#streams) x N cycles, with #streams =
ceil(C/128) * ceil(WIN/HOP) per frame-column. C=514 needs 5 channel
tiles, the 5th holding only 2 channels -- 25% wasted PE time. The device
computes channels 0..511 (4 full tiles = the bf16 PE floor of 16
streams/frame-column ~= 130us/core at the 2.37 GHz top pstate); the host
computes the last 2 channels with one small BLAS GEMM over the strided
frame view (<1% of the FLOPs, valid for any weight values). Device
output is stored bf16 (halves the dominant output DMA: 39.3 -> 19.7
MB/core; rel err 2.1e-3 -> 3.4e-3 vs the 2e-2 gate) and upcast on host.

Host prep: pad x by 300 on both sides, transpose per batch to
XT[b, r, f'] = x_padded[b, 100 f' + r] so device input DMAs move whole
[100, cols] panels with multi-KB contiguous lines; weights passed as
wt[j, r, c] = weight[c, 0, 100 j + r] (independent [100, 512] j-planes).

Device kernel (Bass/Tile): the t = 100j + r tap decomposition (j 0..3,
r 0..99) turns the overlapped strided conv into 4 PSUM-accumulated
matmuls per group:
    out[c, f] = sum_j sum_r wt[j, r, c] * XT[r, f + j]
lhsT = wt[j][:, c-tile] (K=100, M=128), rhs = XT[:, f-tile] (N<=512),
fp32 PSUM accumulation over j with all 8 banks in flight; PSUM is
evacuated alternately by DVE/ACT (casting to bf16) into SBUF rows
[128, 4803], stored in four column pieces (the last only ~50 KB).

Startup (all measured on this hw): engines boot staggered within a NEFF
exec (sync ~0.2us, gpsimd ~5.8, scalar/ACT ~7.1, vector/DVE ~7.4), and
first-DMA completions cannot beat ~10us (queue boot + descriptor pacing
+ completion latency). So: the first XT panel rides the sync ring, the
four weight j-planes fan out over scalar/gpsimd/gpsimd/sync, and the PE
warmup source is a DVE memset (fastest PE start, ~7.9us). The warmup
matmuls MUST be K=128 and span ~3.4us of sustained activity: K=128
activity is what trips the PE clock boost (1.2 -> 2.4 GHz); K=100 work
neither lifts nor re-lifts it, and an unramped stream runs 2.4x slower.
Once lifted, the K=100 stream holds the boost. Measured ~149.5us/core =
~11.8us startup + ~131.5us PE stream (PE >99% busy within its window) +
~3.4us store drain + ~3.2us fixed epilogue; baseline was 183.3us."""

import numpy as np
import ml_dtypes

WIN, HOP, C = 400, 100, 514
C_DEV = 512                           # channels computed on device
B, T = 32, 480000
PAD = WIN - HOP                       # 300
N_CORES = 8
B_LOC = B // N_CORES                  # 4
T_PAD = T + 2 * PAD                   # 480600
N_FRAMES = (T_PAD - WIN) // HOP + 1   # 4803
N_CHUNKS = 4864                       # padded frame columns (128-aligned)
NJ = WIN // HOP                       # 4

F_TILE = 512
C_TILE = 128
FIRST_COLS = 640                      # first XT panel (critical load)
N_WARMUP = 12
STORE_EVERY = 3                       # ftile groups per output store piece


def build_program(b_loc=B_LOC, n_chunks=N_CHUNKS, n_frames=N_FRAMES):
    import concourse.bacc as bacc
    import concourse.mybir as mybir
    import concourse.tile as tile

    dt = mybir.dt
    assert n_frames + NJ - 1 <= n_chunks

    nc = bacc.Bacc("TRN2", target_bir_lowering=False, debug=False)
    x_d = nc.dram_tensor(
        "x", [b_loc, HOP, n_chunks], dt.bfloat16, kind="ExternalInput"
    ).ap()
    w_d = nc.dram_tensor(
        "wt", [NJ, HOP, C_DEV], dt.bfloat16, kind="ExternalInput"
    ).ap()
    o_d = nc.dram_tensor(
        "out", [b_loc, C_DEV, n_frames], dt.bfloat16, kind="ExternalOutput"
    ).ap()

    ctiles = [(c0, min(C_TILE, C_DEV - c0)) for c0 in range(0, C_DEV, C_TILE)]
    ftiles = [(f0, min(F_TILE, n_frames - f0)) for f0 in range(0, n_frames, F_TILE)]
    n_ft = len(ftiles)
    store_at = {}
    lo = 0
    for fi in range(STORE_EVERY - 1, n_ft - 1, STORE_EVERY):
        hi = ftiles[fi][0] + ftiles[fi][1]
        store_at[fi] = (lo, hi)
        lo = hi
    store_at[n_ft - 1] = (lo, n_frames)

    with tile.TileContext(nc) as tc:
        with (
            tc.tile_pool(name="const", bufs=1) as constp,
            tc.tile_pool(name="xt", bufs=2) as xtp,
            tc.tile_pool(name="orow", bufs=7) as orowp,
            tc.tile_pool(name="mmps", bufs=8, space="PSUM") as mmps,
        ):
            xt0 = xtp.tile([HOP, n_chunks], dt.bfloat16, tag="xt")
            nc.sync.dma_start(xt0[:, 0:FIRST_COLS], x_d[0, :, 0:FIRST_COLS])
            wsb = constp.tile([HOP, NJ, C_DEV], dt.bfloat16)
            for j, eng in enumerate((nc.scalar, nc.gpsimd, nc.gpsimd, nc.sync)):
                eng.dma_start(wsb[:, j, :], w_d[j])

            warm = constp.tile([128, 512], dt.bfloat16)
            nc.vector.memset(warm[:], 0.0)
            wps = mmps.tile([128, F_TILE], dt.float32, tag="ps")
            for _ in range(N_WARMUP):
                nc.tensor.matmul(wps[0:16, :], warm[:, 0:16], warm[:])

            for g0 in range(FIRST_COLS, n_chunks, 1056):
                gs = min(1056, n_chunks - g0)
                nc.scalar.dma_start(xt0[:, g0 : g0 + gs], x_d[0, :, g0 : g0 + gs])

            ncopy = 0

            def mm_group(xt, orow, c0, cm, f0, fn):
                nonlocal ncopy
                ps = mmps.tile([128, F_TILE], dt.float32, tag="ps")
                for j in range(NJ):
                    nc.tensor.matmul(
                        ps[0:cm, 0:fn],
                        wsb[0:HOP, j, c0 : c0 + cm],
                        xt[0:HOP, f0 + j : f0 + j + fn],
                        start=(j == 0),
                        stop=(j == NJ - 1),
                    )
                if ncopy % 2 == 1:
                    nc.scalar.copy(orow[0:cm, f0 : f0 + fn], ps[0:cm, 0:fn])
                else:
                    nc.vector.tensor_copy(orow[0:cm, f0 : f0 + fn], ps[0:cm, 0:fn])
                ncopy += 1

            for b in range(b_loc):
                if b == 0:
                    xt = xt0
                else:
                    xt = xtp.tile([HOP, n_chunks], dt.bfloat16, tag="xt")
                    for g0 in range(0, n_chunks, 1216):
                        gs = min(1216, n_chunks - g0)
                        nc.scalar.dma_start(
                            xt[:, g0 : g0 + gs], x_d[b, :, g0 : g0 + gs]
                        )

                for c0, cm in ctiles:
                    orow = orowp.tile(
                        [128, n_frames], dt.bfloat16, tag="orow", name=f"or_{b}_{c0}"
                    )
                    for fi, (f0, fn) in enumerate(ftiles):
                        mm_group(xt, orow, c0, cm, f0, fn)
                        if fi in store_at:
                            slo, shi = store_at[fi]
                            nc.sync.dma_start(
                                o_d[b, c0 : c0 + cm, slo:shi], orow[0:cm, slo:shi]
                            )

    nc.compile()
    return nc


_NC = None
LAST_RESULTS = None


def _ensure_axon_hooks_stub():
    import sys

    try:
        import antenv.axon_hooks  # noqa: F401
    except ImportError:
        import types

        import antenv

        m = types.ModuleType("antenv.axon_hooks")
        m.get_axon_ntff_profile_hook = lambda: None
        m.set_axon_ntff_profile_hook = lambda h: None
        sys.modules["antenv.axon_hooks"] = m
        antenv.axon_hooks = m


def _prep_inputs(x, weight):
    x = np.asarray(x, dtype=np.float32)
    w = np.asarray(weight, dtype=np.float32)
    nb = x.shape[0]
    xp = np.zeros((nb, N_CHUNKS * HOP), dtype=np.float32)
    xp[:, PAD : PAD + x.shape[1]] = x
    xdev = np.ascontiguousarray(
        xp.reshape(nb, N_CHUNKS, HOP).transpose(0, 2, 1)
    ).astype(ml_dtypes.bfloat16)
    wt = np.ascontiguousarray(
        w.reshape(C, WIN)[:C_DEV].T.reshape(NJ, HOP, C_DEV)
    ).astype(ml_dtypes.bfloat16)
    return xp, xdev, wt


def _host_tail_channels(xp, w):
    w2 = np.ascontiguousarray(
        np.asarray(w, dtype=np.float32).reshape(C, WIN)[C_DEV:].T
    )
    v = np.lib.stride_tricks.sliding_window_view(xp, WIN, axis=1)[:, ::HOP, :]
    v = v[:, :N_FRAMES]
    out2 = np.tensordot(v, w2, axes=([2], [0]))
    return np.ascontiguousarray(out2.transpose(0, 2, 1))


def kernel(x, weight):
    global _NC, LAST_RESULTS
    from concourse.bass_utils import run_bass_kernel_spmd

    _ensure_axon_hooks_stub()
    xp, xdev, wt = _prep_inputs(x, weight)
    tail = _host_tail_channels(xp, weight)
    if _NC is None:
        _NC = build_program()
    in_maps = [
        {"x": np.ascontiguousarray(xdev[c * B_LOC : (c + 1) * B_LOC]), "wt": wt}
        for c in range(N_CORES)
    ]
    res = run_bass_kernel_spmd(_NC, in_maps, core_ids=list(range(N_CORES)))
    LAST_RESULTS = res
    out = np.empty((B, C, N_FRAMES), dtype=np.float32)
    for c in range(N_CORES):
        out[c * B_LOC : (c + 1) * B_LOC, :C_DEV] = res.results[c]["out"]
    out[:, C_DEV:] = tail
    return out
